# revision 1
# baseline (speedup 1.0000x reference)
"""Banked linear (MoE routing) kernel for 8 Trainium2 NeuronCores.

Problem: out[b,s,k,:] = tensor[b,s,k,:] @ weight[sel[b,s,k]].T + bias[sel[b,s,k]]
Shapes: tensor (2,256,2,512), sel (2,256,2) int, weight (16,512,512), bias (16,512).

Strategy (expert-parallel, host-routed dispatch):
  * Flatten to 1024 token-slots; group them by selected bank on the host
    (the "all-to-all" of the sharding hint, done during input sharding).
  * 16 banks -> 8 cores, 2 banks per core. Each core reads only its own
    2 banks' weights (16 MiB of weights read exactly once across the chip).
  * Per bank: tokens padded to capacity C, x transposed host-side so the
    device does   psum[C,512] = sum_k xT[k*128:+128, :C].T @ WT[k*128:+128, :512]
  * Outputs scattered back to (B,S,K,OUT) positions on the host; the bias
    gather/add rides along with the scatter (O(out) host work).

General-case fallback: if a bank attracts more than 128 token-slots the
bank is split into several jobs of <=128 tokens (weights re-read per job).
"""

import numpy as np

import concourse.bacc as bacc
import concourse.bass as bass
import concourse.mybir as mybir
import concourse.tile as tile
from concourse.bass_utils import run_bass_kernel_spmd

B, S, K = 2, 256, 2
IN, OUT, NB = 512, 512, 16
N_CORES = 8
P = 128  # partition dim / contraction tile

_MODULES: dict = {}  # (jobs_per_core, capacity) -> compiled bass module
LAST_RESULTS = None  # BassKernelResults of the most recent run (for test.py)


MM_DT = mybir.dt.float32  # full-precision matmul (f32r would be ~13% faster
                          # at ~1.3e-4 rel err; fp32 keeps 1.3e-7)
WARMUP_MMS = 14  # full-width dummy matmuls warm HAM to 2.4 GHz (-0.9 us)
DMA_SCRATCH = 16384  # Bass dynamic_dma_scratch_size
SPLIT_COPY = False  # gpsimd cannot read PSUM; only DVE does this well
W_SPLIT = "ksync"  # weight DMA granularity: "k" | "expert" | "half" | "ksync"
OUT_RING = "sync"   # ring for output DMAs: "sync" | "scalar"
NO_PARTITION_ID = True  # skip partition-id preamble machinery


def _build_module(jpc: int, cap: int) -> bass.Bass:
    f32 = mybir.dt.float32
    mdt = MM_DT
    kt = IN // P
    nc = bacc.Bacc(None, target_bir_lowering=False, debug=False,
                   enable_partition_id=not NO_PARTITION_ID,
                   dynamic_dma_scratch_size=DMA_SCRATCH)
    # x pre-swizzled host-side to [p, j, k, t] so this DMA is contiguous
    xt = nc.dram_tensor("xt", (P, jpc, kt, cap), mdt, kind="ExternalInput")
    # weights pre-swizzled host-side to [j, p, k, n]: contiguous per job
    wt = nc.dram_tensor("wt", (jpc, P, kt, OUT), mdt, kind="ExternalInput")
    out = nc.dram_tensor("out", (jpc, cap, OUT), f32, kind="ExternalOutput")
    dbg = (nc.dram_tensor("dbg", (1, 1), f32, kind="ExternalOutput")
           if WARMUP_MMS else None)

    with tile.TileContext(nc) as tc:
        with (
            tc.tile_pool(name="wp", bufs=jpc) as wp,
            tc.tile_pool(name="xp", bufs=1) as xp,
            tc.tile_pool(name="op", bufs=2) as op,
            tc.tile_pool(name="warm", bufs=1) as wmp,
            tc.tile_pool(name="ps", bufs=2, space="PSUM") as pp,
            tc.tile_pool(name="pswarm", bufs=1, space="PSUM") as ppw,
        ):
            # all jobs' x in one contiguous DMA, first on the scalar ring
            xsb = xp.tile([P, jpc, kt, cap], mdt)
            nc.scalar.dma_start(xsb[:], xt[:])
            # per-job weights; DMA granularity/ring assignment per W_SPLIT
            wsb = []
            ring_i = 0
            for j in range(jpc):
                w = wp.tile([P, kt, OUT], mdt)
                if W_SPLIT == "expert":
                    ring = nc.sync if j % 2 == 0 else nc.scalar
                    ring.dma_start(w[:], wt[j])
                elif W_SPLIT == "half":
                    h = kt // 2
                    nc.sync.dma_start(w[:, :h, :], wt[j, :, :h, :])
                    nc.scalar.dma_start(w[:, h:, :], wt[j, :, h:, :])
                elif W_SPLIT == "ksync":
                    # all weight tiles on the sync ring, consumption order
                    for k in range(kt):
                        nc.sync.dma_start(w[:, k, :], wt[j, :, k, :])
                else:  # "k": one DMA per k-tile, alternating rings
                    for k in range(kt):
                        ring = nc.sync if ring_i % 2 == 0 else nc.scalar
                        ring.dma_start(w[:, k, :], wt[j, :, k, :])
                        ring_i += 1
                wsb.append(w)
            # PE warm-up spin: full-width dummy bf16 matmuls while the
            # weight DMAs are in flight (HAM un-throttles after ~3.4 us of
            # genuine array activity; a 1-wide matmul does not count)
            if WARMUP_MMS:
                bf16 = mybir.dt.bfloat16
                wz = wmp.tile([P, P + OUT], bf16)
                nc.vector.memset(wz[:], 0.0)
                wps = ppw.tile([P, OUT], f32)
                for _ in range(WARMUP_MMS):
                    nc.tensor.matmul(wps[:], wz[:, :P], wz[:, P:],
                                     start=True, stop=True)
                # tiny consumer so the spin isn't dead-code-eliminated
                wdbg = wmp.tile([1, 1], f32)
                nc.vector.tensor_copy(wdbg[:], wps[:1, :1])
                nc.scalar.dma_start(dbg[:], wdbg[:])
            for j in range(jpc):
                if SPLIT_COPY:
                    # two 256-col PSUM groups: first half's copy+store
                    # overlaps the second half's matmuls, shrinking the
                    # post-PE tail
                    h = OUT // 2
                    for half in range(2):
                        psum = pp.tile([cap, h], f32, tag=f"ps{half}")
                        lo = half * h
                        for k in range(kt):
                            nc.tensor.matmul(
                                psum[:], xsb[:, j, k, :],
                                wsb[j][:, k, lo:lo + h],
                                start=(k == 0), stop=(k == kt - 1))
                        osb = op.tile([cap, h], f32, tag=f"os{half}")
                        nc.vector.tensor_copy(osb[:], psum[:])
                        ring = nc.scalar if half == 0 else nc.sync
                        ring.dma_start(out[j, :, lo:lo + h], osb[:])
                else:
                    psum = pp.tile([cap, OUT], f32)
                    for k in range(kt):
                        nc.tensor.matmul(psum[:], xsb[:, j, k, :],
                                         wsb[j][:, k, :],
                                         start=(k == 0), stop=(k == kt - 1))
                    # bias is added host-side on scatter
                    osb = op.tile([cap, OUT], f32)
                    nc.vector.tensor_copy(osb[:], psum[:])
                    out_ring = nc.sync if OUT_RING == "sync" else nc.scalar
                    out_ring.dma_start(out[j], osb[:])
    nc.compile()
    return nc


def _get_module(jpc: int, cap: int) -> bass.Bass:
    key = (jpc, cap)
    if key not in _MODULES:
        _MODULES[key] = _build_module(jpc, cap)
    return _MODULES[key]


def kernel(tensor, bank_selections, weight, bias):
    global LAST_RESULTS
    tensor = np.asarray(tensor, dtype=np.float32)
    out_shape = tensor.shape[:-1] + (OUT,)
    x = np.ascontiguousarray(tensor.reshape(-1, IN))
    sel = np.asarray(bank_selections).reshape(-1).astype(np.int64)
    weight = np.asarray(weight, dtype=np.float32)
    bias = np.asarray(bias, dtype=np.float32)
    n_tok = sel.shape[0]

    order = np.argsort(sel, kind="stable")
    counts = np.bincount(sel, minlength=NB)
    starts = np.concatenate(([0], np.cumsum(counts)))

    # jobs: (bank, token index array), each <= 128 tokens
    jobs = []
    for e in range(NB):
        idx = order[starts[e]:starts[e + 1]]
        if len(idx) <= P:
            jobs.append((e, idx))
        else:
            for lo in range(0, len(idx), P):
                jobs.append((e, idx[lo:lo + P]))
    # pad job count to a multiple of N_CORES
    while len(jobs) % N_CORES:
        jobs.append((0, np.empty(0, np.int64)))
    jpc = len(jobs) // N_CORES
    cap = max(16, -(-max(len(idx) for _, idx in jobs) // 16) * 16)

    kt = IN // P
    XT = np.zeros((N_CORES, jpc, kt, P, cap), np.float32)
    WT = np.empty((N_CORES, jpc, kt, P, OUT), np.float32)
    for j, (e, idx) in enumerate(jobs):
        c, s = j % N_CORES, j // N_CORES
        if len(idx):
            XT[c, s].reshape(IN, cap)[:, :len(idx)] = x[idx].T
        WT[c, s] = weight[e].T.reshape(kt, P, OUT)
    # device wants x as [p, j, k, t] and w as [j, p, k, n], both contiguous
    np_dt = mybir.dt.np(MM_DT)
    XT = np.ascontiguousarray(XT.transpose(0, 3, 1, 2, 4), dtype=np_dt)
    WT = np.ascontiguousarray(WT.transpose(0, 1, 3, 2, 4), dtype=np_dt)

    nc = _get_module(jpc, cap)
    in_maps = [{"xt": XT[c], "wt": WT[c]} for c in range(N_CORES)]
    res = run_bass_kernel_spmd(nc, in_maps, core_ids=list(range(N_CORES)))
    LAST_RESULTS = res

    out_full = np.empty((n_tok, OUT), np.float32)
    for j, (e, idx) in enumerate(jobs):
        if not len(idx):
            continue
        c, s = j % N_CORES, j // N_CORES
        out_full[idx] = res.results[c]["out"][s, :len(idx)] + bias[e]
    return out_full.reshape(out_shape)



# revision 4
# speedup vs baseline: 1.1715x; 1.1715x over previous
"""Banked linear (MoE routing) kernel for 8 Trainium2 NeuronCores.

Problem: out[b,s,k,:] = tensor[b,s,k,:] @ weight[sel[b,s,k]].T + bias[sel[b,s,k]]
Shapes: tensor (2,256,2,512), sel (2,256,2) int, weight (16,512,512), bias (16,512).

Strategy (expert-parallel, host-routed dispatch):
  * Flatten to 1024 token-slots; group them by selected bank on the host
    (the "all-to-all" of the sharding hint, done during input sharding).
  * 16 banks -> 8 cores, 2 banks per core. Each core reads only its own
    2 banks' weights (8 MiB of bf16 weights read exactly once across the chip).
  * Everything on the wire is bf16 (weights, x, out) -- the PE runs bf16
    matmuls at full rate (fp32 needs LOW/HIGH double passes) and DMA bytes
    halve; rel err ~3e-3 vs the 2e-2 gate.
  * Weights live contiguous per job in DRAM, so each job is one dense
    512 KB DMA at near-peak HBM bandwidth; x rides first on the sync ring.
  * A few full-width warm-up matmuls run while the weights stream in,
    ramping the HAM clock so the real matmuls run at full rate.
  * Per bank: tokens padded to capacity C, x transposed host-side so the
    device does   psum[C,512] = sum_k xT[k*128:+128, :C].T @ WT[k*128:+128, :512]
  * The tail PSUM drain (bf16 cast) is split across DVE and ACT, and the
    final job's output goes out as two half-width DMAs on separate HWDGE
    rings so the HBM write-receipt tails overlap.
  * Outputs scattered back to (B,S,K,OUT) positions on the host; the bias
    gather/add rides along with the scatter (O(out) host work).

General-case fallback: if a bank attracts more than 128 token-slots the
bank is split into several jobs of <=128 tokens (weights re-read per job).
"""

import os

import numpy as np

import concourse.bacc as bacc
import concourse.bass as bass
import concourse.mybir as mybir
import concourse.tile as tile
from concourse.bass_utils import run_bass_kernel_spmd

B, S, K = 2, 256, 2
IN, OUT, NB = 512, 512, 16
N_CORES = 8
P = 128  # partition dim / contraction tile

_MODULES: dict = {}  # (jobs_per_core, capacity) -> compiled bass module
LAST_RESULTS = None  # BassKernelResults of the most recent run (for test.py)


def _env(name, default):
    v = os.environ.get(name)
    return type(default)(v) if v is not None else default


MM_DT = mybir.dt.bfloat16  # wire + matmul dtype
OUT_DT = mybir.dt.bfloat16  # output wire dtype (cast on PSUM->SBUF drain)
WARMUP_MMS = _env("KN_WARMUP", 6)  # PE clock-ramp spin during the w stream
W_ONE_DMA = _env("KN_WONE", 0)  # 1: all jobs' weights in a single DMA
SPLIT_LAST = _env("KN_SPLITLAST", 1)  # split final job's drain DVE+ACT
DMA_SCRATCH = 16384
NO_PARTITION_ID = True


def _build_module(jpc: int, cap: int) -> bass.Bass:
    f32 = mybir.dt.float32
    mdt = MM_DT
    kt = IN // P
    nc = bacc.Bacc(None, target_bir_lowering=False, debug=False,
                   enable_partition_id=not NO_PARTITION_ID,
                   dynamic_dma_scratch_size=DMA_SCRATCH)
    # x pre-swizzled host-side to [p, j, k, t] so this DMA is contiguous
    xt = nc.dram_tensor("xt", (P, jpc, kt, cap), mdt, kind="ExternalInput")
    # weights pre-swizzled host-side to [j, p, k, n]: contiguous per job
    wt = nc.dram_tensor("wt", (jpc, P, kt, OUT), mdt, kind="ExternalInput")
    out = nc.dram_tensor("out", (jpc, cap, OUT), OUT_DT, kind="ExternalOutput")
    dbg = (nc.dram_tensor("dbg", (1, 1), f32, kind="ExternalOutput")
           if WARMUP_MMS else None)

    with tile.TileContext(nc) as tc:
        with (
            tc.tile_pool(name="wp", bufs=jpc) as wp,
            tc.tile_pool(name="xp", bufs=1) as xp,
            tc.tile_pool(name="op", bufs=2) as op,
            tc.tile_pool(name="warm", bufs=1) as wmp,
            tc.tile_pool(name="ps", bufs=2, space="PSUM") as pp,
            tc.tile_pool(name="pswarm", bufs=1, space="PSUM") as ppw,
        ):
            # all jobs' x in one contiguous DMA, first in the sync FIFO
            xsb = xp.tile([P, jpc, kt, cap], mdt)
            nc.sync.dma_start(xsb[:], xt[:])
            # per-job weights: one dense contiguous DMA per job
            wsb = []
            if W_ONE_DMA:
                wall = wp.tile([jpc, P, kt, OUT], mdt)
                nc.sync.dma_start(wall[:], wt[:])
                wsb = [wall[j] for j in range(jpc)]
            else:
                for j in range(jpc):
                    w = wp.tile([P, kt, OUT], mdt)
                    rg = nc.sync if j % 2 == 0 else nc.scalar
                    rg.dma_start(w[:], wt[j])
                    wsb.append(w)
            # PE warm-up spin overlapping the weight stream: ramps the HAM
            # clock (2x matmul rate) before the real matmuls need it
            if WARMUP_MMS:
                bf16 = mybir.dt.bfloat16
                wz = wmp.tile([P, P + OUT], bf16)
                nc.vector.memset(wz[:], 0.0)
                wps = ppw.tile([P, OUT], f32)
                for _ in range(WARMUP_MMS):
                    nc.tensor.matmul(wps[:], wz[:, :P], wz[:, P:],
                                     start=True, stop=True)
                # tiny consumer so the spin isn't dead-code-eliminated
                wdbg = wmp.tile([1, 1], f32)
                nc.vector.tensor_copy(wdbg[:], wps[:1, :1])
                nc.gpsimd.dma_start(dbg[:], wdbg[:])
            h = OUT // 2
            for j in range(jpc):
                psum = pp.tile([cap, OUT], f32)
                for k in range(kt):
                    nc.tensor.matmul(psum[:], xsb[:, j, k, :],
                                     wsb[j][:, k, :],
                                     start=(k == 0), stop=(k == kt - 1))
                # bias is added host-side on scatter
                osb = op.tile([cap, OUT], OUT_DT)
                if SPLIT_LAST and j == jpc - 1:
                    # tail drain: DVE and ACT each take half, outputs go
                    # out on both rings so the receipt tails overlap
                    nc.vector.tensor_copy(osb[:, :h], psum[:, :h])
                    nc.scalar.copy(osb[:, h:], psum[:, h:])
                    nc.sync.dma_start(out[j, :, :h], osb[:, :h])
                    nc.scalar.dma_start(out[j, :, h:], osb[:, h:])
                else:
                    nc.vector.tensor_copy(osb[:], psum[:])
                    nc.sync.dma_start(out[j], osb[:])
    nc.compile()
    return nc


def _get_module(jpc: int, cap: int) -> bass.Bass:
    key = (jpc, cap)
    if key not in _MODULES:
        _MODULES[key] = _build_module(jpc, cap)
    return _MODULES[key]


def kernel(tensor, bank_selections, weight, bias):
    global LAST_RESULTS
    tensor = np.asarray(tensor, dtype=np.float32)
    out_shape = tensor.shape[:-1] + (OUT,)
    x = np.ascontiguousarray(tensor.reshape(-1, IN))
    sel = np.asarray(bank_selections).reshape(-1).astype(np.int64)
    weight = np.asarray(weight, dtype=np.float32)
    bias = np.asarray(bias, dtype=np.float32)
    n_tok = sel.shape[0]

    order = np.argsort(sel, kind="stable")
    counts = np.bincount(sel, minlength=NB)
    starts = np.concatenate(([0], np.cumsum(counts)))

    # jobs: (bank, token index array), each <= 128 tokens
    jobs = []
    for e in range(NB):
        idx = order[starts[e]:starts[e + 1]]
        if len(idx) <= P:
            jobs.append((e, idx))
        else:
            for lo in range(0, len(idx), P):
                jobs.append((e, idx[lo:lo + P]))
    # pad job count to a multiple of N_CORES
    while len(jobs) % N_CORES:
        jobs.append((0, np.empty(0, np.int64)))
    jpc = len(jobs) // N_CORES
    cap = max(16, -(-max(len(idx) for _, idx in jobs) // 16) * 16)

    kt = IN // P
    XT = np.zeros((N_CORES, jpc, kt, P, cap), np.float32)
    WT = np.empty((N_CORES, jpc, kt, P, OUT), np.float32)
    for j, (e, idx) in enumerate(jobs):
        c, s = j % N_CORES, j // N_CORES
        if len(idx):
            XT[c, s].reshape(IN, cap)[:, :len(idx)] = x[idx].T
        WT[c, s] = weight[e].T.reshape(kt, P, OUT)
    # device wants x as [p, j, k, t] and w as [j, p, k, n], both contiguous
    np_dt = mybir.dt.np(MM_DT)
    XT = np.ascontiguousarray(XT.transpose(0, 3, 1, 2, 4), dtype=np_dt)
    WT = np.ascontiguousarray(WT.transpose(0, 1, 3, 2, 4), dtype=np_dt)

    nc = _get_module(jpc, cap)
    in_maps = [{"xt": XT[c], "wt": WT[c]} for c in range(N_CORES)]
    res = run_bass_kernel_spmd(nc, in_maps, core_ids=list(range(N_CORES)))
    LAST_RESULTS = res

    out_full = np.empty((n_tok, OUT), np.float32)
    for j, (e, idx) in enumerate(jobs):
        if not len(idx):
            continue
        c, s = j % N_CORES, j // N_CORES
        job_out = np.asarray(res.results[c]["out"][s, :len(idx)],
                             dtype=np.float32)
        out_full[idx] = job_out + bias[e]
    return out_full.reshape(out_shape)


# revision 17
# speedup vs baseline: 1.3924x; 1.1885x over previous
"""Banked linear (MoE routing) kernel for 8 Trainium2 NeuronCores.

Problem: out[b,s,k,:] = tensor[b,s,k,:] @ weight[sel[b,s,k]].T + bias[sel[b,s,k]]
Shapes: tensor (2,256,2,512), sel (2,256,2) int, weight (16,512,512), bias (16,512).

Strategy (expert-parallel, host-routed dispatch):
  * Flatten to 1024 token-slots; group them by selected bank on the host
    (the "all-to-all" of the sharding hint, done during input sharding).
  * 16 banks -> 8 cores, 2 banks per core. Each core reads only its own
    2 banks' weights (8 MiB of bf16 weights read exactly once across the chip).
  * Everything on the wire is bf16 (weights, x, out) -- the PE runs bf16
    matmuls at full rate (fp32 needs LOW/HIGH double passes at 2x the
    time) and DMA bytes halve; rel err ~3e-3 vs the 2e-2 gate.
  * Weight DMAs go out in 256 KB (2 k-tile) chunks, job0 on the sync
    HWDGE ring, job1 on the scalar ring, so matmuls start as soon as the
    first chunk + x land (~230 GB/s effective aggregate stream).
  * Six full-width bf16 warm-up matmuls (operand memset on the otherwise
    idle GpSimd) spin the PE while the weights stream in: the HAM clock
    un-throttles after ~3.4 us of array activity, so the real matmuls run
    at 216 ns instead of 427 ns.  They target job0's psum tile, which the
    real k=0 matmul (start=True) overwrites -- no consumer needed.
  * Per bank: tokens padded to capacity C, x transposed host-side so the
    device does   psum[C,512] = sum_k xT[k*128:+128, :C].T @ WT[k*128:+128, :512]
  * PSUM drains cast f32->bf16 on the fly; the final job's drain runs on
    ACT (slightly faster than DVE and not queued behind job0's DVE drain)
    and its output DMA rides the otherwise-idle scalar ring.
  * Outputs scattered back to (B,S,K,OUT) positions on the host; the bias
    gather/add rides along with the scatter (O(out) host work).

Measured on the 8-core axon pod: ~18.3-19.0 us exec (from 25.5 us
baseline).  Of that, ~8 us is fixed NEFF overhead (a ~6 us epilogue that
clears all 256 semaphores serially per engine, plus start barriers), ~5.4
us is the 1.2 MiB/core weight+x stream, and the rest is drain/out/receipt
tail -- all three near their floors for this decomposition.

General-case fallback: if a bank attracts more than 128 token-slots the
bank is split into several jobs of <=128 tokens (weights re-read per job).
"""

import os

import numpy as np

import concourse.bacc as bacc
import concourse.bass as bass
import concourse.mybir as mybir
import concourse.tile as tile
from concourse.bass_utils import run_bass_kernel_spmd

B, S, K = 2, 256, 2
IN, OUT, NB = 512, 512, 16
N_CORES = 8
P = 128  # partition dim / contraction tile

_MODULES: dict = {}  # (jobs_per_core, capacity) -> compiled bass module
LAST_RESULTS = None  # BassKernelResults of the most recent run (for test.py)


def _env(name, default):
    v = os.environ.get(name)
    return type(default)(v) if v is not None else default


MM_DT = mybir.dt.bfloat16  # wire + matmul dtype
OUT_DT = mybir.dt.bfloat16  # output wire dtype (cast on PSUM->SBUF drain)
WARMUP_MMS = _env("KN_WARMUP", 6)  # PE clock-ramp spin during the w stream
W_KT_CHUNK = _env("KN_WKT", 2)  # k-tiles per weight DMA
SPLIT_LAST = _env("KN_SPLITLAST", 0)  # split final job's drain DVE+ACT
SPLIT_ALL = _env("KN_SPLITALL", 0)  # split every job's drain DVE+ACT
PACK_XW = _env("KN_PACK", 0)  # pack x and w into combined per-k-pair DMAs
WARM_PSUM = _env("KN_WARMPSUM", 1)  # warm-ups write job0's psum (no dbg)
ACT_LAST = _env("KN_ACTLAST", 1)  # final job's drain on ACT, out on scalar
X_RING = os.environ.get("KN_XRING", "scalar")
OUT_RING = os.environ.get("KN_OUTRING", "sync")
DMA_SCRATCH = 16384
NO_PARTITION_ID = True


def _build_module(jpc: int, cap: int) -> bass.Bass:
    f32 = mybir.dt.float32
    mdt = MM_DT
    kt = IN // P
    m = cap + OUT  # packed x||w row length per (job, k)
    nc = bacc.Bacc(None, target_bir_lowering=False, debug=False,
                   enable_partition_id=not NO_PARTITION_ID,
                   dynamic_dma_scratch_size=DMA_SCRATCH)
    if PACK_XW:
        # x and w packed host-side to [j, p, k, cap+OUT]: each (job, k-pair)
        # chunk is one DMA whose bytes arrive in exact consumption order
        xwt = nc.dram_tensor("xwt", (jpc, P, kt, m), mdt,
                             kind="ExternalInput")
    else:
        # x pre-swizzled host-side to [p, j, k, t] so this DMA is contiguous
        xt = nc.dram_tensor("xt", (P, jpc, kt, cap), mdt,
                            kind="ExternalInput")
        # weights pre-swizzled host-side to [j, p, k, n]: contiguous per job
        wt = nc.dram_tensor("wt", (jpc, P, kt, OUT), mdt,
                            kind="ExternalInput")
    out = nc.dram_tensor("out", (jpc, cap, OUT), OUT_DT, kind="ExternalOutput")
    dbg = (nc.dram_tensor("dbg", (1, 1), f32, kind="ExternalOutput")
           if WARMUP_MMS and not WARM_PSUM else None)

    def ring(name):
        return {"sync": nc.sync, "scalar": nc.scalar}[name]

    with tile.TileContext(nc) as tc:
        with (
            tc.tile_pool(name="wp", bufs=jpc) as wp,
            tc.tile_pool(name="xp", bufs=1) as xp,
            tc.tile_pool(name="op", bufs=2) as op,
            tc.tile_pool(name="warm", bufs=1) as wmp,
            tc.tile_pool(name="ps", bufs=2, space="PSUM") as pp,
            tc.tile_pool(name="pswarm", bufs=1, space="PSUM") as ppw,
        ):
            if PACK_XW:
                xw = []
                for j in range(jpc):
                    t = wp.tile([P, kt, m], mdt)
                    xw.append(t)
                # interleave issue: (j0,k01) (j1,k01) (j0,k23) (j1,k23)
                for k0 in range(0, kt, W_KT_CHUNK):
                    for j in range(jpc):
                        rg = ring("sync" if j % 2 == 0 else "scalar")
                        rg.dma_start(xw[j][:, k0:k0 + W_KT_CHUNK, :],
                                     xwt[j, :, k0:k0 + W_KT_CHUNK, :])
                xsb_of = lambda j, k: xw[j][:, k, :cap]
                wsb_of = lambda j, k: xw[j][:, k, cap:]
            else:
                # all jobs' x in one contiguous DMA
                xsb = xp.tile([P, jpc, kt, cap], mdt)
                ring(X_RING).dma_start(xsb[:], xt[:])
                # per-job weights, W_KT_CHUNK k-tiles per DMA, ring per job
                wsb = []
                for j in range(jpc):
                    w = wp.tile([P, kt, OUT], mdt)
                    rg = ring("sync" if j % 2 == 0 else "scalar")
                    for k0 in range(0, kt, W_KT_CHUNK):
                        rg.dma_start(w[:, k0:k0 + W_KT_CHUNK, :],
                                     wt[j, :, k0:k0 + W_KT_CHUNK, :])
                    wsb.append(w)
                xsb_of = lambda j, k: xsb[:, j, k, :]
                wsb_of = lambda j, k: wsb[j][:, k, :]
            # PE warm-up spin overlapping the weight stream: ramps the HAM
            # clock (2x matmul rate) before the real matmuls need it
            psums = [pp.tile([cap, OUT], f32, tag=f"psum{j}",
                             name=f"psum{j}") for j in range(jpc)]
            if WARMUP_MMS and WARM_PSUM:
                # warm-ups write into job0's psum tile; the real k=0 matmul
                # (start=True) overwrites it, so no separate consumer or
                # psum bank is needed and the spin can't be dead-code'd
                bf16 = mybir.dt.bfloat16
                wz = wmp.tile([P, cap + OUT], bf16)
                nc.gpsimd.memset(wz[:], 0.0)
                for _ in range(WARMUP_MMS):
                    nc.tensor.matmul(psums[0][:], wz[:, :cap], wz[:, cap:],
                                     start=True, stop=True)
            elif WARMUP_MMS:
                bf16 = mybir.dt.bfloat16
                wz = wmp.tile([P, P + OUT], bf16)
                nc.vector.memset(wz[:], 0.0)
                wps = ppw.tile([P, OUT], f32)
                for _ in range(WARMUP_MMS):
                    nc.tensor.matmul(wps[:], wz[:, :P], wz[:, P:],
                                     start=True, stop=True)
                # tiny consumer so the spin isn't dead-code-eliminated
                wdbg = wmp.tile([1, 1], f32)
                nc.vector.tensor_copy(wdbg[:], wps[:1, :1])
                nc.gpsimd.dma_start(dbg[:], wdbg[:])
            h = OUT // 2
            for j in range(jpc):
                psum = psums[j]
                for k in range(kt):
                    nc.tensor.matmul(psum[:], xsb_of(j, k), wsb_of(j, k),
                                     start=(k == 0), stop=(k == kt - 1))
                # bias is added host-side on scatter
                osb = op.tile([cap, OUT], OUT_DT)
                if ACT_LAST and j == jpc - 1:
                    # tail drain entirely on ACT (faster than DVE for this
                    # shape and frees it from queuing behind job0's drain);
                    # out rides the otherwise-idle scalar ring
                    nc.scalar.copy(osb[:], psum[:])
                    nc.scalar.dma_start(out[j], osb[:])
                elif SPLIT_ALL or (SPLIT_LAST and j == jpc - 1):
                    # drain split: DVE and ACT each take half, outputs go
                    # out on both rings so the receipt tails overlap
                    nc.vector.tensor_copy(osb[:, :h], psum[:, :h])
                    nc.scalar.copy(osb[:, h:], psum[:, h:])
                    nc.sync.dma_start(out[j, :, :h], osb[:, :h])
                    nc.scalar.dma_start(out[j, :, h:], osb[:, h:])
                else:
                    nc.vector.tensor_copy(osb[:], psum[:])
                    ring(OUT_RING).dma_start(out[j], osb[:])
    nc.compile()
    return nc


def _get_module(jpc: int, cap: int) -> bass.Bass:
    key = (jpc, cap)
    if key not in _MODULES:
        _MODULES[key] = _build_module(jpc, cap)
    return _MODULES[key]


def kernel(tensor, bank_selections, weight, bias):
    global LAST_RESULTS
    tensor = np.asarray(tensor, dtype=np.float32)
    out_shape = tensor.shape[:-1] + (OUT,)
    x = np.ascontiguousarray(tensor.reshape(-1, IN))
    sel = np.asarray(bank_selections).reshape(-1).astype(np.int64)
    weight = np.asarray(weight, dtype=np.float32)
    bias = np.asarray(bias, dtype=np.float32)
    n_tok = sel.shape[0]

    order = np.argsort(sel, kind="stable")
    counts = np.bincount(sel, minlength=NB)
    starts = np.concatenate(([0], np.cumsum(counts)))

    # jobs: (bank, token index array), each <= 128 tokens
    jobs = []
    for e in range(NB):
        idx = order[starts[e]:starts[e + 1]]
        if len(idx) <= P:
            jobs.append((e, idx))
        else:
            for lo in range(0, len(idx), P):
                jobs.append((e, idx[lo:lo + P]))
    # pad job count to a multiple of N_CORES
    while len(jobs) % N_CORES:
        jobs.append((0, np.empty(0, np.int64)))
    jpc = len(jobs) // N_CORES
    cap = max(16, -(-max(len(idx) for _, idx in jobs) // 16) * 16)

    kt = IN // P
    XT = np.zeros((N_CORES, jpc, kt, P, cap), np.float32)
    WT = np.empty((N_CORES, jpc, kt, P, OUT), np.float32)
    for j, (e, idx) in enumerate(jobs):
        c, s = j % N_CORES, j // N_CORES
        if len(idx):
            XT[c, s].reshape(IN, cap)[:, :len(idx)] = x[idx].T
        WT[c, s] = weight[e].T.reshape(kt, P, OUT)
    np_dt = mybir.dt.np(MM_DT)
    nc = _get_module(jpc, cap)
    if PACK_XW:
        # device wants one [j, p, k, cap+OUT] tensor with x||w interleaved
        XW = np.empty((N_CORES, jpc, P, kt, cap + OUT), np.float32)
        XW[:, :, :, :, :cap] = XT.transpose(0, 1, 3, 2, 4)
        XW[:, :, :, :, cap:] = WT.transpose(0, 1, 3, 2, 4)
        XW = np.ascontiguousarray(XW, dtype=np_dt)
        in_maps = [{"xwt": XW[c]} for c in range(N_CORES)]
    else:
        # device wants x as [p, j, k, t] and w as [j, p, k, n], contiguous
        XTd = np.ascontiguousarray(XT.transpose(0, 3, 1, 2, 4), dtype=np_dt)
        WTd = np.ascontiguousarray(WT.transpose(0, 1, 3, 2, 4), dtype=np_dt)
        in_maps = [{"xt": XTd[c], "wt": WTd[c]} for c in range(N_CORES)]
    res = run_bass_kernel_spmd(nc, in_maps, core_ids=list(range(N_CORES)))
    LAST_RESULTS = res

    out_full = np.empty((n_tok, OUT), np.float32)
    for j, (e, idx) in enumerate(jobs):
        if not len(idx):
            continue
        c, s = j % N_CORES, j // N_CORES
        job_out = np.asarray(res.results[c]["out"][s, :len(idx)],
                             dtype=np.float32)
        out_full[idx] = job_out + bias[e]
    return out_full.reshape(out_shape)
